# revision 50
# baseline (speedup 1.0000x reference)
"""BatchTopK SAE Trainium2 kernel (8 NeuronCores, SPMD data-parallel).

Algorithm (per core c, batch rows 256c..256c+255):
  encode:  post.T[f, m] = relu(W_enc @ (x - b_dec).T + b_enc) via split-bf16x3
           GEMM (hi/lo decomposition, fp32 PSUM accumulate) -- matches fp32
           reference to ~1e-6 while running the PE at bf16 rate.
           Fused per f-tile: ACT computes relu + per-partition value sums
           (for the sigma estimate), DVE extracts top-8 per dictionary row
           (single max8) as threshold candidates.
           The threshold prep is folded INTO the encode loop: a provisional
           sigma from the first 32 f-tiles fixes the bracket [lo0, hi0];
           as each 8-tile segment of L1 completes it is counted (exact
           #>=hi0 / #>=lo0), band-filtered and compacted (top-8/segment),
           so when the last f-tile retires only the sidecars + AllGather
           remain.
  topk:    the global batch top-(K*B) reduces to a scalar threshold; each
           core AllGathers its 128 compacted candidates + sidecars, then
           runs an identical branch-free fp32 false-position iteration
           (6 rounds) on the gathered array.  The hi-side bracket is taken
           as the final threshold: count(>= hi) converges to within ~2 of
           K*B (verified in simulation), far inside the error budget.
  decode:  x_hat = (post * (post >= t)) @ W_dec.T + b_dec with bf16 masked
           activations / weights (value error ~0.2%).  Decoder weight tiles
           are prefetched during the threshold phase so the decode GEMM is
           PE-bound, not DMA-bound.

Everything runs in ONE SPMD launch; host only reshapes inputs and concats
the per-core [256, 768] output slices.
"""

import numpy as np

ACT_DIM = 768
DICT = 16384
K = 64
BATCH = 2048
NCORES = 8
ROWS = BATCH // NCORES        # 256 batch rows per core
FT = DICT // 128              # 128 dictionary tiles
DT = ACT_DIM // 128           # 6 contraction tiles
MT = ROWS // 128              # 2 output row tiles
L1_W = FT * 8                 # 1024 level-1 candidate cols (top-8/dict row)
NSEG = 16                     # level-2 segments (64 L1 cols = 8 f-tiles each)
SEGW = L1_W // NSEG           # 64
L2_W = NSEG * 8               # 128 level-2 candidate cols
PAY = L2_W + 8                # gather payload (sidecars in cols 128..131)
SIGT = 32                     # f-tiles used for the provisional sigma
CTARGET = float(K * BATCH)    # 131072
NSECANT = 4
NPREF = 13                    # decode weight tile-pairs prefetched during topk

import os as _os
ENC = _os.environ.get("SAE_ENC", "bf16")         # fp8 | bf16
NXTERM = 3                                       # fp8 split terms per operand
# product passes (i, j) grouped by combined scale 16^-(i+j); passes with
# i+j >= 3 are dropped (noise ~4e-5 of sigma, ~20 top-k flips, verified)
FP8_CHAINS = [
    [(0, 0)],
    [(0, 1), (1, 0)],
    [(1, 1), (0, 2), (2, 0)],
]
if _os.environ.get("SAE_FP8_PASSES", "6") == "8":
    FP8_CHAINS.append([(1, 2), (2, 1)])
WSCALE = 32.0                                    # pre-scale of W into fp8 range
FP8_CHAIN_SCALE = [16.0 ** -(c) / WSCALE for c in range(len(FP8_CHAINS))]
# bf16 mode: the W_lo correction chain runs as fp8 DoubleRow (half the
# matmuls); its operands carry only ~5e-4 of the signal so fp8's 3.6%
# relative noise is ~2e-5 absolute (verified end-to-end: rel err 0.0076)
BSCALE = 8192.0

# Bracket constants: t* = sigma * z * (1 + model error); margins +-1.5%
# verified offline against two datasets (model error observed <= +0.42%,
# sigma sampling 3sd from 1M samples 0.17%).
_Z = 2.66007 * 1.002
A_LO = float(np.float32(_Z * 0.985))
A_HI = float(np.float32(_Z * 1.015))
SIG_SCALE = float(np.float32(np.sqrt(2.0 * np.pi) / (SIGT * 128 * ROWS)))


def build_nc():
    from concourse import bass, bacc, mybir, tile, bass_isa

    dt = mybir.dt
    Alu = mybir.AluOpType
    nc = bacc.Bacc(num_devices=NCORES)

    # ---- DRAM I/O ----
    if ENC == "fp8":
        x8 = nc.dram_tensor("x8", [128, NXTERM, DT, ROWS], dt.float8e4, kind="ExternalInput")
        w8 = nc.dram_tensor("w8", [FT, 128, NXTERM, DT, 128], dt.float8e4, kind="ExternalInput")
    else:
        xt_hi = nc.dram_tensor("xt_hi", [128, DT, ROWS], dt.bfloat16, kind="ExternalInput")
        xt_lo8 = nc.dram_tensor("xt_lo8", [128, DT, ROWS], dt.float8e4, kind="ExternalInput")
        x8h = nc.dram_tensor("x8h", [128, DT, ROWS], dt.float8e4, kind="ExternalInput")
        wenc_hi = nc.dram_tensor("wenc_hi", [FT, 128, DT, 128], dt.bfloat16, kind="ExternalInput")
        wenc_lo8 = nc.dram_tensor("wenc_lo8", [FT, 128, DT, 128], dt.float8e4, kind="ExternalInput")
        wenc_hi8 = nc.dram_tensor("wenc_hi8", [FT, 128, DT, 128], dt.float8e4, kind="ExternalInput")
    wdect = nc.dram_tensor("wdect", [FT // 2, 128, 2, ACT_DIM], dt.bfloat16, kind="ExternalInput")
    benc = nc.dram_tensor("benc", [128, FT], dt.float32, kind="ExternalInput")
    bdec_b = nc.dram_tensor("bdec_b", [128, ACT_DIM], dt.float32, kind="ExternalInput")
    xhat = nc.dram_tensor("xhat", [ROWS, ACT_DIM], dt.float32, kind="ExternalOutput")

    with tile.TileContext(nc) as tc:
        with (
            tc.tile_pool(name="persist", bufs=1) as P,
            tc.tile_pool(name="dram", bufs=1, space="DRAM") as D,
        ):
            post = P.tile([128, FT * ROWS], dt.float32, tag="post")
            l1 = P.tile([128, L1_W], dt.float32, tag="l1")
            sums = P.tile([128, FT], dt.float32, tag="sums")
            if ENC == "fp8":
                x8_s = P.tile([128, NXTERM, DT, ROWS], dt.float8e4, tag="x8")
            else:
                xh_s = P.tile([128, DT, ROWS], dt.bfloat16, tag="xh")
                xl8_s = P.tile([128, DT, ROWS], dt.float8e4, tag="xl8")
                x8h_s = P.tile([128, DT, ROWS], dt.float8e4, tag="x8h")
            benc_s = P.tile([128, FT], dt.float32, tag="benc")
            bdec_s = P.tile([128, ACT_DIM], dt.float32, tag="bdec")
            l2 = P.tile([128, PAY], dt.float32, tag="l2")
            gath = P.tile([128, NCORES, PAY], dt.float32, tag="gath")
            cscr_a = P.tile([128, 2 * SEGW], dt.float32, tag="cscr_a")
            cscr_b = P.tile([128, NCORES, L2_W], dt.float32, tag="cscr_b")
            chp_cols = P.tile([128, NSEG], dt.float32, tag="chp_cols")
            clp_cols = P.tile([128, NSEG], dt.float32, tag="clp_cols")
            ones_t = P.tile([128, 128], dt.float32, tag="ones_t")
            g_in = D.tile([128, PAY], dt.float32)
            g_out = D.tile([NCORES, 128, PAY], dt.float32, addr_space="Shared")

            # scalar state tiles [128, 1]
            def sc(tag):
                return P.tile([128, 1], dt.float32, tag=tag, name=tag)

            sig = sc("sig"); lo0 = sc("lo0"); hi0 = sc("hi0")
            lo = sc("lo"); hi = sc("hi"); clo = sc("clo"); chi = sc("chi")
            chg = sc("chg"); t = sc("t"); ct = sc("ct"); cp = sc("cp")
            pred = sc("pred"); npred = sc("npred")
            inv = sc("inv"); tmp1 = sc("tmp1"); tmp2 = sc("tmp2"); tmp3 = sc("tmp3")

            if ENC == "fp8":
                nc.sync.dma_start(out=x8_s[:], in_=x8[:])
            else:
                nc.sync.dma_start(out=xh_s[:], in_=xt_hi[:])
                nc.sync.dma_start(out=xl8_s[:], in_=xt_lo8[:])
                nc.sync.dma_start(out=x8h_s[:], in_=x8h[:])
            nc.scalar.dma_start(out=benc_s[:], in_=benc[:])
            nc.scalar.dma_start(out=bdec_s[:], in_=bdec_b[:])
            nc.vector.memset(ones_t[:], 1.0)
            nc.vector.memset(l2[:, L2_W + 4:PAY], 0.0)
            # warm up the gpsimd/Q7 engine so the collective trigger at the
            # end of encode doesn't pay its cold-start
            nc.gpsimd.memset(cscr_a[:, 0:1], 0.0)

            tt = nc.vector.tensor_tensor
            ts = nc.vector.tensor_scalar
            stt = nc.vector.scalar_tensor_tensor

            with tc.tile_pool(name="rpsum", bufs=2, space="PSUM") as RP:

                def psum_reduce(in_ap, out_ap, scale=None, add_ap=None):
                    # cross-partition sum+broadcast via PE: out[p] = sum_q in[q]
                    rps = RP.tile([128, 1], dt.float32, tag="rps", name="rps")
                    nc.tensor.matmul(rps[:], ones_t[:], in_ap, start=True, stop=True)
                    if scale is not None:
                        nc.vector.tensor_scalar_mul(out_ap, rps[:], scale)
                    elif add_ap is not None:
                        nc.vector.tensor_add(out_ap, rps[:], add_ap)
                    else:
                        nc.vector.tensor_copy(out_ap, rps[:])

                def seg_compact(s):
                    # exact counts vs bracket, band filter, top-8 compaction
                    seg = l1[:, s * SEGW:(s + 1) * SEGW]
                    ts(cscr_a[:, 0:SEGW], seg, hi0[:], None,
                       op0=Alu.is_ge, op1=Alu.add, accum_out=chp_cols[:, s:s + 1])
                    ts(cscr_a[:, SEGW:2 * SEGW], seg, lo0[:], None,
                       op0=Alu.is_ge, op1=Alu.add, accum_out=clp_cols[:, s:s + 1])
                    stt(seg, seg, hi0[:], seg, op0=Alu.is_lt, op1=Alu.mult)
                    nc.vector.max(out=l2[:, s * 8:s * 8 + 8], in_=seg)

                # ================= encode =================
                # fp8 path: x and W split into 3 e4m3 terms each (W pre-scaled
                # by 32); product passes grouped into PSUM chains by combined
                # scale 16^-(i+j) and run as DoubleRow matmuls (2 contraction
                # subtiles per instruction, 0.5 cycles/row) -- half the PE
                # cycles of the bf16x3 split.  DVE folds the scaled chains.
                NCH = len(FP8_CHAINS)
                with (
                    tc.tile_pool(name="wenc", bufs=5) as WP,
                    tc.tile_pool(name="epsum", bufs=3, space="PSUM") as EP,
                    tc.tile_pool(name="epsumb", bufs=3, space="PSUM") as EPB,
                    tc.tile_pool(name="efold", bufs=3) as EF,
                ):
                    # the fp8 correction weights ride the scalar DGE queue,
                    # issued 3 tiles ahead (the ACT engine's stream trails the
                    # PE by ~a tile, so triggers must lead); bf16 main weights
                    # stream on the sync queue
                    fp8w = {}

                    def issue_fp8w(ft):
                        wel = WP.tile([128, DT, 128], dt.float8e4, tag="wel")
                        nc.scalar.dma_start(out=wel[:], in_=wenc_lo8[ft])
                        wh8 = WP.tile([128, DT, 128], dt.float8e4, tag="wh8")
                        nc.scalar.dma_start(out=wh8[:], in_=wenc_hi8[ft])
                        fp8w[ft] = (wel, wh8)

                    if ENC != "fp8":
                        for ft in range(3):
                            issue_fp8w(ft)
                    for ft in range(FT):
                        if ENC == "fp8":
                            w8t = WP.tile([128, NXTERM, DT, 128], dt.float8e4, tag="w8t")
                            eng = nc.sync if ft % 2 else nc.scalar
                            eng.dma_start(out=w8t[:], in_=w8[ft])
                            psc = EP.tile([128, NCH, ROWS], dt.float32, tag="eps")
                            for c, chain in enumerate(FP8_CHAINS):
                                n_mm = 3 * len(chain)
                                m = 0
                                for (i, j) in chain:
                                    for k in range(DT // 2):
                                        nc.tensor.matmul(
                                            psc[:, c, :],
                                            w8t[:, j, 2 * k:2 * k + 2, :],
                                            x8_s[:, i, 2 * k:2 * k + 2, :],
                                            start=(m == 0), stop=(m == n_mm - 1),
                                            perf_mode=mybir.MatmulPerfMode.DoubleRow,
                                        )
                                        m += 1
                            fold = EF.tile([128, ROWS], dt.float32, tag="fold")
                            nc.vector.tensor_scalar_mul(
                                fold[:], psc[:, 0, :], FP8_CHAIN_SCALE[0])
                            for c in range(1, NCH):
                                stt(fold[:], psc[:, c, :], FP8_CHAIN_SCALE[c],
                                    fold[:], op0=Alu.mult, op1=Alu.add)
                        else:
                            if ft + 3 < FT:
                                issue_fp8w(ft + 3)
                            wel, wh8 = fp8w.pop(ft)
                            weh = WP.tile([128, DT, 128], dt.bfloat16, tag="weh")
                            nc.sync.dma_start(out=weh[:], in_=wenc_hi[ft])
                            # main term bf16; both fp8 correction chains carry
                            # the same 8192x pre-scale, so they share one PSUM
                            # accumulation (W_lo.x + W_hi.x_lo)
                            ps_a = EP.tile([128, ROWS], dt.float32, tag="eps")
                            ps_bc = EPB.tile([128, ROWS], dt.float32, tag="epsb")
                            for d in range(DT):
                                nc.tensor.matmul(
                                    ps_a[:], weh[:, d, :], xh_s[:, d, :],
                                    start=(d == 0), stop=(d == DT - 1),
                                )
                            for k in range(DT // 2):
                                nc.tensor.matmul(
                                    ps_bc[:], wel[:, 2 * k:2 * k + 2, :],
                                    x8h_s[:, 2 * k:2 * k + 2, :],
                                    start=(k == 0), stop=False,
                                    perf_mode=mybir.MatmulPerfMode.DoubleRow,
                                )
                            for k in range(DT // 2):
                                nc.tensor.matmul(
                                    ps_bc[:], wh8[:, 2 * k:2 * k + 2, :],
                                    xl8_s[:, 2 * k:2 * k + 2, :],
                                    start=False, stop=(k == DT // 2 - 1),
                                    perf_mode=mybir.MatmulPerfMode.DoubleRow,
                                )
                            fold = EF.tile([128, ROWS], dt.float32, tag="fold")
                            nc.vector.tensor_copy(fold[:], ps_a[:])
                            stt(fold[:], ps_bc[:], 1.0 / BSCALE, fold[:],
                                op0=Alu.mult, op1=Alu.add)
                        pslice = post[:, ft * ROWS:(ft + 1) * ROWS]
                        nc.scalar.activation(
                            out=pslice,
                            in_=fold[:],
                            func=mybir.ActivationFunctionType.Relu,
                            bias=benc_s[:, ft:ft + 1],
                            scale=1.0,
                            accum_out=sums[:, ft:ft + 1],
                        )
                        # L1 candidates: top-8 of each dictionary row
                        nc.vector.max(out=l1[:, ft * 8:ft * 8 + 8], in_=pslice)

                        if ft == SIGT - 1:
                            # provisional sigma -> bracket [lo0, hi0]
                            nc.vector.tensor_reduce(
                                out=tmp1[:], in_=sums[:, 0:SIGT],
                                axis=mybir.AxisListType.X, op=Alu.add)
                            psum_reduce(tmp1[:], sig[:], scale=SIG_SCALE)
                            nc.vector.tensor_scalar_mul(lo0[:], sig[:], A_LO)
                            nc.vector.tensor_scalar_mul(hi0[:], sig[:], A_HI)
                            for s in range(SIGT // 8):
                                seg_compact(s)
                        elif ft >= SIGT and (ft + 1) % 8 == 0:
                            seg_compact((ft + 1) // 8 - 1)

                # prefetch decode weights; the DMAs drain during the topk phase
                with (
                    tc.tile_pool(name="wdec", bufs=NPREF + 2) as WD,
                    tc.tile_pool(name="dpsum", bufs=2, space="PSUM") as DP,
                    tc.tile_pool(name="msk", bufs=5) as MS,
                    tc.tile_pool(name="outs", bufs=2) as OS,
                ):
                    # ================= threshold =================
                    nc.vector.tensor_reduce(out=tmp1[:], in_=chp_cols[:],
                                            axis=mybir.AxisListType.X, op=Alu.add)
                    psum_reduce(tmp1[:], tmp2[:])
                    nc.vector.tensor_copy(l2[:, L2_W + 2:L2_W + 3], tmp2[:])
                    nc.vector.tensor_reduce(out=tmp1[:], in_=clp_cols[:],
                                            axis=mybir.AxisListType.X, op=Alu.add)
                    psum_reduce(tmp1[:], tmp3[:])
                    nc.vector.tensor_copy(l2[:, L2_W + 3:L2_W + 4], tmp3[:])
                    nc.vector.tensor_copy(l2[:, L2_W:L2_W + 1], lo0[:])
                    nc.vector.tensor_copy(l2[:, L2_W + 1:L2_W + 2], hi0[:])

                    # AllGather input first in the scalar DGE queue (it only
                    # waits on l2's writers), decode-weight prefetch behind it
                    nc.scalar.dma_start(out=g_in[:], in_=l2[:])
                    wd_tiles = {}
                    for fp in range(NPREF):
                        wd2 = WD.tile([128, 2, ACT_DIM], dt.bfloat16, tag="wd")
                        nc.scalar.dma_start(out=wd2[:], in_=wdect[fp])
                        wd_tiles[fp] = wd2
                    nc.gpsimd.collective_compute(
                        "AllGather",
                        Alu.bypass,
                        replica_groups=[list(range(NCORES))],
                        ins=[g_in.opt()],
                        outs=[g_out.opt()],
                    )
                    for c in range(NCORES):
                        nc.sync.dma_start(out=gath[:, c, :], in_=g_out[c])

                    gv = gath[:, :, 0:L2_W]

                    # global bracket / counts from sidecars
                    nc.vector.tensor_reduce(out=lo[:], in_=gath[:, :, L2_W:L2_W + 1],
                                            axis=mybir.AxisListType.XY, op=Alu.max)
                    nc.vector.tensor_reduce(out=hi[:], in_=gath[:, :, L2_W + 1:L2_W + 2],
                                            axis=mybir.AxisListType.XY, op=Alu.min)
                    nc.vector.tensor_reduce(out=chg[:], in_=gath[:, :, L2_W + 2:L2_W + 3],
                                            axis=mybir.AxisListType.XY, op=Alu.add)
                    nc.vector.tensor_reduce(out=clo[:], in_=gath[:, :, L2_W + 3:L2_W + 4],
                                            axis=mybir.AxisListType.XY, op=Alu.add)

                    def count_ge(t_ap, out_ap):
                        ts(cscr_b[:], gv, t_ap, None,
                           op0=Alu.is_ge, op1=Alu.add, accum_out=cp[:])
                        psum_reduce(cp[:], out_ap, add_ap=chg[:])

                    # exact count at the hi bracket; clo stays the (approximate)
                    # sidecar sum -- it only steers the first interpolation
                    count_ge(hi[:], chi[:])

                    # branch-free false position; the hi side converges onto the
                    # target count from below and is the final threshold
                    for it in range(NSECANT):
                        tt(tmp1[:], clo[:], chi[:], op=Alu.subtract)
                        nc.vector.tensor_scalar_max(tmp1[:], tmp1[:], 1.0)
                        nc.vector.reciprocal(inv[:], tmp1[:])
                        ts(tmp2[:], chi[:], -1.0, CTARGET, op0=Alu.mult, op1=Alu.add)
                        tt(tmp2[:], tmp2[:], inv[:], op=Alu.mult)
                        tt(tmp3[:], lo[:], hi[:], op=Alu.subtract)
                        stt(t[:], tmp3[:], tmp2[:], hi[:], op0=Alu.mult, op1=Alu.add)
                        count_ge(t[:], ct[:])
                        ts(pred[:], ct[:], CTARGET, None, op0=Alu.is_ge)
                        ts(npred[:], pred[:], -1.0, 1.0, op0=Alu.mult, op1=Alu.add)
                        tt(tmp1[:], t[:], lo[:], op=Alu.subtract)
                        stt(lo[:], tmp1[:], pred[:], lo[:], op0=Alu.mult, op1=Alu.add)
                        tt(tmp1[:], ct[:], clo[:], op=Alu.subtract)
                        stt(clo[:], tmp1[:], pred[:], clo[:], op0=Alu.mult, op1=Alu.add)
                        tt(tmp1[:], t[:], hi[:], op=Alu.subtract)
                        stt(hi[:], tmp1[:], npred[:], hi[:], op0=Alu.mult, op1=Alu.add)
                        tt(tmp1[:], ct[:], chi[:], op=Alu.subtract)
                        stt(chi[:], tmp1[:], npred[:], chi[:], op0=Alu.mult, op1=Alu.add)

                    # ================= decode =================
                    HA = ACT_DIM // 2  # 384 -- one matmul per PSUM bank
                    pso = [
                        DP.tile([128, 2, 512], dt.float32, tag="dps", name=f"dps{mt}")
                        for mt in range(MT)
                    ]
                    for fp in range(FT // 2):
                        wd2 = wd_tiles.pop(fp, None)
                        if wd2 is None:
                            # alternate the two hardware DGE queues: aggregate
                            # weight-stream bandwidth, not one queue's cap
                            wd2 = WD.tile([128, 2, ACT_DIM], dt.bfloat16, tag="wd")
                            eng = nc.scalar if fp % 2 else nc.sync
                            eng.dma_start(out=wd2[:], in_=wdect[fp])
                        for half in range(2):
                            ft = 2 * fp + half
                            pslice = post[:, ft * ROWS:(ft + 1) * ROWS]
                            mskt = MS.tile([128, ROWS], dt.bfloat16, tag="mskt")
                            stt(mskt[:], pslice, hi[:], pslice, op0=Alu.is_ge, op1=Alu.mult)
                            for mt in range(MT):
                                for h in range(2):
                                    nc.tensor.matmul(
                                        pso[mt][:, h, 0:HA],
                                        mskt[:, mt * 128:(mt + 1) * 128],
                                        wd2[:, half, h * HA:(h + 1) * HA],
                                        start=(ft == 0),
                                        stop=(ft == FT - 1),
                                    )
                    for mt in range(MT):
                        outs = OS.tile([128, ACT_DIM], dt.float32, tag="outs")
                        for h in range(2):
                            nc.vector.tensor_add(
                                outs[:, h * HA:(h + 1) * HA],
                                pso[mt][:, h, 0:HA],
                                bdec_s[:, h * HA:(h + 1) * HA],
                            )
                        nc.sync.dma_start(out=xhat[mt * 128:(mt + 1) * 128, :], in_=outs[:])

    nc.finalize()
    return nc


def _split_fp8(a, nterms):
    """a -> [t0, t1, ...] e4m3 terms with t_i ~ (residual * 16^i); the true
    value is sum_i t_i / 16^i."""
    import ml_dtypes
    f8 = ml_dtypes.float8_e4m3fn
    terms = []
    r = a.astype(np.float32)
    for i in range(nterms):
        t = (r * np.float32(16.0 ** i)).astype(f8)
        terms.append(t)
        r = r - t.astype(np.float32) / np.float32(16.0 ** i)
    return terms


def _prep_inputs(x, W_enc, b_enc, W_dec, b_dec):
    import ml_dtypes
    bf16 = ml_dtypes.bfloat16

    x0T = np.ascontiguousarray(
        (x.astype(np.float32) - b_dec.astype(np.float32)[None, :]).T
    )  # [768, 2048]
    WT = np.ascontiguousarray(W_enc.astype(np.float32).T)  # [768, 16384]

    def wlay(a):  # [768, 16384] -> [FT, 128(p=d), DT, 128(f)]
        return np.ascontiguousarray(
            a.reshape(DT, 128, FT, 128).transpose(2, 1, 0, 3)
        )

    WdT = (
        np.ascontiguousarray(W_dec.astype(np.float32).T).astype(bf16)
        .reshape(FT // 2, 2, 128, ACT_DIM).transpose(0, 2, 1, 3).copy()
    )  # [64, 128, 2, 768]: tile pairs share one DMA line
    bencL = np.ascontiguousarray(b_enc.astype(np.float32).reshape(FT, 128).T)
    bdecB = np.ascontiguousarray(
        np.broadcast_to(b_dec.astype(np.float32)[None, :], (128, ACT_DIM))
    )

    common = {"wdect": WdT, "benc": bencL, "bdec_b": bdecB}
    if ENC == "fp8":
        xterms = _split_fp8(x0T, NXTERM)             # each [768, 2048]
        wterms = _split_fp8(WT * np.float32(WSCALE), NXTERM)  # each [768, 16384]
        # w8: [FT, 128(p=d), NXTERM, DT, 128(f)]
        w8 = np.stack([wlay(t) for t in wterms], axis=2)
        common["w8"] = np.ascontiguousarray(w8)
        xl_ = [t.reshape(DT, 128, BATCH).transpose(1, 0, 2) for t in xterms]
    else:
        import ml_dtypes
        f8 = ml_dtypes.float8_e4m3fn
        XLS = np.float32(256.0)
        WHS = np.float32(BSCALE / 256.0)  # 32; XLS*WHS == BSCALE
        xh = x0T.astype(bf16)
        xl8 = ((x0T - xh.astype(np.float32)) * XLS).astype(f8)
        x8h_full = x0T.astype(f8)
        Wh = WT.astype(bf16)
        Wl8 = ((WT - Wh.astype(np.float32)) * np.float32(BSCALE)).astype(f8)
        Wh8 = (WT * WHS).astype(f8)
        common["wenc_hi"] = wlay(Wh)
        common["wenc_lo8"] = wlay(Wl8)
        common["wenc_hi8"] = wlay(Wh8)

    in_maps = []
    for c in range(NCORES):
        sl = slice(c * ROWS, (c + 1) * ROWS)
        m = dict(common)
        if ENC == "fp8":
            # x8: [128(p=d), NXTERM, DT, ROWS]
            m["x8"] = np.ascontiguousarray(
                np.stack([t[:, :, sl] for t in xl_], axis=1)
            )
        else:
            m["xt_hi"] = np.ascontiguousarray(
                xh[:, sl].reshape(DT, 128, ROWS).transpose(1, 0, 2))
            m["xt_lo8"] = np.ascontiguousarray(
                xl8[:, sl].reshape(DT, 128, ROWS).transpose(1, 0, 2))
            m["x8h"] = np.ascontiguousarray(
                x8h_full[:, sl].reshape(DT, 128, ROWS).transpose(1, 0, 2))
        in_maps.append(m)
    return in_maps


def _ensure_axon_hooks_shim():
    """concourse's trace path imports antenv.axon_hooks, which some images
    lack; install an equivalent module so tracing degrades (or works, when
    the ctypes hook is available) instead of crashing."""
    import sys, types
    try:
        import antenv.axon_hooks  # noqa: F401
        return
    except ImportError:
        pass
    m = types.ModuleType("antenv.axon_hooks")
    state = {"hook": None}
    m.set_axon_ntff_profile_hook = lambda h: state.__setitem__("hook", h)
    m.get_axon_ntff_profile_hook = lambda: state["hook"]
    sys.modules["antenv.axon_hooks"] = m
    try:
        from trn_agent_boot.trn_boot import _ntff_profile_via_ctypes
        hook = _ntff_profile_via_ctypes("/opt/axon/libaxon_pjrt.so")
        if hook is not None:
            m.set_axon_ntff_profile_hook(hook)
    except Exception:
        pass


def kernel(x, W_enc, b_enc, W_dec, b_dec):
    import os
    _ensure_axon_hooks_shim()
    from concourse import bass_utils
    from concourse.bass_utils import run_bass_kernel_spmd

    in_maps = _prep_inputs(x, W_enc, b_enc, W_dec, b_dec)
    nc = build_nc()
    res = None
    if os.environ.get("KERNEL_TRACE"):
        bass_utils.upload_artifacts = lambda d: ""  # no artifact bucket here
        try:
            res = run_bass_kernel_spmd(nc, in_maps, list(range(NCORES)), trace=True)
        except Exception as e:
            print(f"traced run failed ({type(e).__name__}: {e}); retrying untraced")
            res = None
    if res is None:
        res = run_bass_kernel_spmd(nc, in_maps, list(range(NCORES)))
    if res.exec_time_ns is not None:
        print(f"HW exec time: {res.exec_time_ns} ns")
    out = np.concatenate(
        [np.asarray(res.results[c]["xhat"], dtype=np.float32) for c in range(NCORES)],
        axis=0,
    )
    return out


# revision 51
# speedup vs baseline: 1.0431x; 1.0431x over previous
"""BatchTopK SAE Trainium2 kernel (8 NeuronCores, SPMD data-parallel).

Algorithm (per core c, batch rows 256c..256c+255):
  encode:  post.T[f, m] = relu(W_enc @ (x - b_dec).T + b_enc).  Main term
           bf16 (W_hi.x_hi, 6 matmuls/f-tile); the two rounding-correction
           terms (W_lo.x and W_hi.x_lo, each ~5e-4 of signal) run as fp8
           e4m3 DoubleRow matmuls (2 contraction subtiles/instr, 3 matmuls
           each) sharing one pre-scaled PSUM chain -- 3072 PE rows/f-tile
           vs 4608 for the bf16x3 split, at ~2e-5 extra noise (end-to-end
           rel err 0.0100 vs gate 0.02, deterministic on the fixed data).
           Fused per f-tile: ACT computes relu + per-partition value sums
           (for the sigma estimate), DVE extracts top-8 per dictionary row
           (single max8) as threshold candidates.
           The threshold prep is folded INTO the encode loop: a provisional
           sigma from the first 32 f-tiles fixes the bracket [lo0, hi0];
           as each 8-tile segment of L1 completes it is counted (exact
           #>=hi0 / #>=lo0), band-filtered and compacted (top-8/segment),
           so when the last f-tile retires only the sidecars + AllGather
           remain.
  topk:    the global batch top-(K*B) reduces to a scalar threshold; each
           core AllGathers its 128 compacted candidates + sidecars, then
           runs an identical branch-free fp32 false-position iteration
           (4 rounds) on the gathered array.  The hi-side bracket is taken
           as the final threshold: count(>= hi) converges to within ~8 of
           K*B (verified in simulation), far inside the error budget.
  decode:  x_hat = (post * (post >= t)) @ W_dec.T + b_dec with bf16 masked
           activations / weights (value error ~0.2%).  Decoder weight tiles
           travel in pairs (3KB DMA lines) on both hardware DGE queues and
           are prefetched during the threshold phase.

Everything runs in ONE SPMD launch; host only reshapes inputs and concats
the per-core [256, 768] output slices.
"""

import numpy as np

ACT_DIM = 768
DICT = 16384
K = 64
BATCH = 2048
NCORES = 8
ROWS = BATCH // NCORES        # 256 batch rows per core
FT = DICT // 128              # 128 dictionary tiles
DT = ACT_DIM // 128           # 6 contraction tiles
MT = ROWS // 128              # 2 output row tiles
L1_W = FT * 8                 # 1024 level-1 candidate cols (top-8/dict row)
NSEG = 16                     # level-2 segments (64 L1 cols = 8 f-tiles each)
SEGW = L1_W // NSEG           # 64
L2_W = NSEG * 8               # 128 level-2 candidate cols
PAY = L2_W + 8                # gather payload (sidecars in cols 128..131)
SIGT = 32                     # f-tiles used for the provisional sigma
CTARGET = float(K * BATCH)    # 131072
NSECANT = 4
NPREF = 13                    # decode weight tile-pairs prefetched during topk

import os as _os
ENC = _os.environ.get("SAE_ENC", "bf16")         # fp8 | bf16
NXTERM = 3                                       # fp8 split terms per operand
# product passes (i, j) grouped by combined scale 16^-(i+j); passes with
# i+j >= 3 are dropped (noise ~4e-5 of sigma, ~20 top-k flips, verified)
FP8_CHAINS = [
    [(0, 0)],
    [(0, 1), (1, 0)],
    [(1, 1), (0, 2), (2, 0)],
]
if _os.environ.get("SAE_FP8_PASSES", "6") == "8":
    FP8_CHAINS.append([(1, 2), (2, 1)])
WSCALE = 32.0                                    # pre-scale of W into fp8 range
FP8_CHAIN_SCALE = [16.0 ** -(c) / WSCALE for c in range(len(FP8_CHAINS))]
# bf16 mode: the W_lo correction chain runs as fp8 DoubleRow (half the
# matmuls); its operands carry only ~5e-4 of the signal so fp8's 3.6%
# relative noise is ~2e-5 absolute (verified end-to-end: rel err 0.0076)
BSCALE = 8192.0

# Bracket constants: t* = sigma * z * (1 + model error); margins +-1.5%
# verified offline against two datasets (model error observed <= +0.42%,
# sigma sampling 3sd from 1M samples 0.17%).
_Z = 2.66007 * 1.002
A_LO = float(np.float32(_Z * 0.985))
A_HI = float(np.float32(_Z * 1.015))
SIG_SCALE = float(np.float32(np.sqrt(2.0 * np.pi) / (SIGT * 128 * ROWS)))


def build_nc():
    from concourse import bass, bacc, mybir, tile, bass_isa

    dt = mybir.dt
    Alu = mybir.AluOpType
    nc = bacc.Bacc(num_devices=NCORES)

    # ---- DRAM I/O ----
    if ENC == "fp8":
        x8 = nc.dram_tensor("x8", [128, NXTERM, DT, ROWS], dt.float8e4, kind="ExternalInput")
        w8 = nc.dram_tensor("w8", [FT, 128, NXTERM, DT, 128], dt.float8e4, kind="ExternalInput")
    else:
        xt_hi = nc.dram_tensor("xt_hi", [128, DT, ROWS], dt.bfloat16, kind="ExternalInput")
        xt_lo8 = nc.dram_tensor("xt_lo8", [128, DT, ROWS], dt.float8e4, kind="ExternalInput")
        x8h = nc.dram_tensor("x8h", [128, DT, ROWS], dt.float8e4, kind="ExternalInput")
        wenc_hi = nc.dram_tensor("wenc_hi", [FT, 128, DT, 128], dt.bfloat16, kind="ExternalInput")
        wenc_lo8 = nc.dram_tensor("wenc_lo8", [FT, 128, DT, 128], dt.float8e4, kind="ExternalInput")
        wenc_hi8 = nc.dram_tensor("wenc_hi8", [FT, 128, DT, 128], dt.float8e4, kind="ExternalInput")
    wdect = nc.dram_tensor("wdect", [FT // 2, 128, 2, ACT_DIM], dt.bfloat16, kind="ExternalInput")
    benc = nc.dram_tensor("benc", [128, FT], dt.float32, kind="ExternalInput")
    bdec_b = nc.dram_tensor("bdec_b", [128, ACT_DIM], dt.float32, kind="ExternalInput")
    xhat = nc.dram_tensor("xhat", [ROWS, ACT_DIM], dt.float32, kind="ExternalOutput")

    with tile.TileContext(nc) as tc:
        with (
            tc.tile_pool(name="persist", bufs=1) as P,
            tc.tile_pool(name="dram", bufs=1, space="DRAM") as D,
        ):
            post = P.tile([128, FT * ROWS], dt.float32, tag="post")
            l1 = P.tile([128, L1_W], dt.float32, tag="l1")
            sums = P.tile([128, FT], dt.float32, tag="sums")
            if ENC == "fp8":
                x8_s = P.tile([128, NXTERM, DT, ROWS], dt.float8e4, tag="x8")
            else:
                xh_s = P.tile([128, DT, ROWS], dt.bfloat16, tag="xh")
                xl8_s = P.tile([128, DT, ROWS], dt.float8e4, tag="xl8")
                x8h_s = P.tile([128, DT, ROWS], dt.float8e4, tag="x8h")
            benc_s = P.tile([128, FT], dt.float32, tag="benc")
            bdec_s = P.tile([128, ACT_DIM], dt.float32, tag="bdec")
            l2 = P.tile([128, PAY], dt.float32, tag="l2")
            gath = P.tile([128, NCORES, PAY], dt.float32, tag="gath")
            cscr_a = P.tile([128, 2 * SEGW], dt.float32, tag="cscr_a")
            cscr_b = P.tile([128, NCORES, L2_W], dt.float32, tag="cscr_b")
            chp_cols = P.tile([128, NSEG], dt.float32, tag="chp_cols")
            clp_cols = P.tile([128, NSEG], dt.float32, tag="clp_cols")
            ones_t = P.tile([128, 128], dt.float32, tag="ones_t")
            g_in = D.tile([128, PAY], dt.float32)
            g_out = D.tile([NCORES, 128, PAY], dt.float32, addr_space="Shared")

            # scalar state tiles [128, 1]
            def sc(tag):
                return P.tile([128, 1], dt.float32, tag=tag, name=tag)

            sig = sc("sig"); lo0 = sc("lo0"); hi0 = sc("hi0")
            lo = sc("lo"); hi = sc("hi"); clo = sc("clo"); chi = sc("chi")
            chg = sc("chg"); t = sc("t"); ct = sc("ct"); cp = sc("cp")
            pred = sc("pred"); npred = sc("npred")
            inv = sc("inv"); tmp1 = sc("tmp1"); tmp2 = sc("tmp2"); tmp3 = sc("tmp3")

            if ENC == "fp8":
                nc.sync.dma_start(out=x8_s[:], in_=x8[:])
            else:
                nc.sync.dma_start(out=xh_s[:], in_=xt_hi[:])
                nc.sync.dma_start(out=xl8_s[:], in_=xt_lo8[:])
                nc.sync.dma_start(out=x8h_s[:], in_=x8h[:])
            nc.scalar.dma_start(out=benc_s[:], in_=benc[:])
            nc.scalar.dma_start(out=bdec_s[:], in_=bdec_b[:])
            nc.vector.memset(ones_t[:], 1.0)
            nc.vector.memset(l2[:, L2_W + 4:PAY], 0.0)
            # warm up the gpsimd/Q7 engine so the collective trigger at the
            # end of encode doesn't pay its cold-start
            nc.gpsimd.memset(cscr_a[:, 0:1], 0.0)

            tt = nc.vector.tensor_tensor
            ts = nc.vector.tensor_scalar
            stt = nc.vector.scalar_tensor_tensor

            with tc.tile_pool(name="rpsum", bufs=2, space="PSUM") as RP:

                def psum_reduce(in_ap, out_ap, scale=None, add_ap=None):
                    # cross-partition sum+broadcast via PE: out[p] = sum_q in[q]
                    rps = RP.tile([128, 1], dt.float32, tag="rps", name="rps")
                    nc.tensor.matmul(rps[:], ones_t[:], in_ap, start=True, stop=True)
                    if scale is not None:
                        nc.vector.tensor_scalar_mul(out_ap, rps[:], scale)
                    elif add_ap is not None:
                        nc.vector.tensor_add(out_ap, rps[:], add_ap)
                    else:
                        nc.vector.tensor_copy(out_ap, rps[:])

                def seg_compact(s):
                    # exact counts vs bracket, band filter, top-8 compaction
                    seg = l1[:, s * SEGW:(s + 1) * SEGW]
                    ts(cscr_a[:, 0:SEGW], seg, hi0[:], None,
                       op0=Alu.is_ge, op1=Alu.add, accum_out=chp_cols[:, s:s + 1])
                    ts(cscr_a[:, SEGW:2 * SEGW], seg, lo0[:], None,
                       op0=Alu.is_ge, op1=Alu.add, accum_out=clp_cols[:, s:s + 1])
                    stt(seg, seg, hi0[:], seg, op0=Alu.is_lt, op1=Alu.mult)
                    nc.vector.max(out=l2[:, s * 8:s * 8 + 8], in_=seg)

                # ================= encode =================
                # fp8 path: x and W split into 3 e4m3 terms each (W pre-scaled
                # by 32); product passes grouped into PSUM chains by combined
                # scale 16^-(i+j) and run as DoubleRow matmuls (2 contraction
                # subtiles per instruction, 0.5 cycles/row) -- half the PE
                # cycles of the bf16x3 split.  DVE folds the scaled chains.
                NCH = len(FP8_CHAINS)
                with (
                    tc.tile_pool(name="wenc", bufs=5) as WP,
                    tc.tile_pool(name="epsum", bufs=3, space="PSUM") as EP,
                    tc.tile_pool(name="epsumb", bufs=3, space="PSUM") as EPB,
                    tc.tile_pool(name="efold", bufs=3) as EF,
                ):
                    # the fp8 correction weights ride the scalar DGE queue,
                    # issued 3 tiles ahead (the ACT engine's stream trails the
                    # PE by ~a tile, so triggers must lead); bf16 main weights
                    # stream on the sync queue
                    fp8w = {}

                    def issue_fp8w(ft):
                        wel = WP.tile([128, DT, 128], dt.float8e4, tag="wel")
                        nc.scalar.dma_start(out=wel[:], in_=wenc_lo8[ft])
                        wh8 = WP.tile([128, DT, 128], dt.float8e4, tag="wh8")
                        nc.scalar.dma_start(out=wh8[:], in_=wenc_hi8[ft])
                        fp8w[ft] = (wel, wh8)

                    if ENC != "fp8":
                        for ft in range(3):
                            issue_fp8w(ft)
                    for ft in range(FT):
                        if ENC == "fp8":
                            w8t = WP.tile([128, NXTERM, DT, 128], dt.float8e4, tag="w8t")
                            eng = nc.sync if ft % 2 else nc.scalar
                            eng.dma_start(out=w8t[:], in_=w8[ft])
                            psc = EP.tile([128, NCH, ROWS], dt.float32, tag="eps")
                            for c, chain in enumerate(FP8_CHAINS):
                                n_mm = 3 * len(chain)
                                m = 0
                                for (i, j) in chain:
                                    for k in range(DT // 2):
                                        nc.tensor.matmul(
                                            psc[:, c, :],
                                            w8t[:, j, 2 * k:2 * k + 2, :],
                                            x8_s[:, i, 2 * k:2 * k + 2, :],
                                            start=(m == 0), stop=(m == n_mm - 1),
                                            perf_mode=mybir.MatmulPerfMode.DoubleRow,
                                        )
                                        m += 1
                            fold = EF.tile([128, ROWS], dt.float32, tag="fold")
                            nc.vector.tensor_scalar_mul(
                                fold[:], psc[:, 0, :], FP8_CHAIN_SCALE[0])
                            for c in range(1, NCH):
                                stt(fold[:], psc[:, c, :], FP8_CHAIN_SCALE[c],
                                    fold[:], op0=Alu.mult, op1=Alu.add)
                        else:
                            if ft + 3 < FT:
                                issue_fp8w(ft + 3)
                            wel, wh8 = fp8w.pop(ft)
                            weh = WP.tile([128, DT, 128], dt.bfloat16, tag="weh")
                            nc.sync.dma_start(out=weh[:], in_=wenc_hi[ft])
                            # main term bf16; both fp8 correction chains carry
                            # the same 8192x pre-scale, so they share one PSUM
                            # accumulation (W_lo.x + W_hi.x_lo)
                            ps_a = EP.tile([128, ROWS], dt.float32, tag="eps")
                            ps_bc = EPB.tile([128, ROWS], dt.float32, tag="epsb")
                            for d in range(DT):
                                nc.tensor.matmul(
                                    ps_a[:], weh[:, d, :], xh_s[:, d, :],
                                    start=(d == 0), stop=(d == DT - 1),
                                )
                            for k in range(DT // 2):
                                nc.tensor.matmul(
                                    ps_bc[:], wel[:, 2 * k:2 * k + 2, :],
                                    x8h_s[:, 2 * k:2 * k + 2, :],
                                    start=(k == 0), stop=False,
                                    perf_mode=mybir.MatmulPerfMode.DoubleRow,
                                )
                            for k in range(DT // 2):
                                nc.tensor.matmul(
                                    ps_bc[:], wh8[:, 2 * k:2 * k + 2, :],
                                    xl8_s[:, 2 * k:2 * k + 2, :],
                                    start=False, stop=(k == DT // 2 - 1),
                                    perf_mode=mybir.MatmulPerfMode.DoubleRow,
                                )
                            fold = EF.tile([128, ROWS], dt.float32, tag="fold")
                            nc.vector.tensor_copy(fold[:], ps_a[:])
                            stt(fold[:], ps_bc[:], 1.0 / BSCALE, fold[:],
                                op0=Alu.mult, op1=Alu.add)
                        pslice = post[:, ft * ROWS:(ft + 1) * ROWS]
                        nc.scalar.activation(
                            out=pslice,
                            in_=fold[:],
                            func=mybir.ActivationFunctionType.Relu,
                            bias=benc_s[:, ft:ft + 1],
                            scale=1.0,
                            accum_out=sums[:, ft:ft + 1],
                        )
                        # L1 candidates: top-8 of each dictionary row
                        nc.vector.max(out=l1[:, ft * 8:ft * 8 + 8], in_=pslice)

                        if ft == SIGT - 1:
                            # provisional sigma -> bracket [lo0, hi0]
                            nc.vector.tensor_reduce(
                                out=tmp1[:], in_=sums[:, 0:SIGT],
                                axis=mybir.AxisListType.X, op=Alu.add)
                            psum_reduce(tmp1[:], sig[:], scale=SIG_SCALE)
                            nc.vector.tensor_scalar_mul(lo0[:], sig[:], A_LO)
                            nc.vector.tensor_scalar_mul(hi0[:], sig[:], A_HI)
                            for s in range(SIGT // 8):
                                seg_compact(s)
                        elif ft >= SIGT and (ft + 1) % 8 == 0:
                            seg_compact((ft + 1) // 8 - 1)

                # prefetch decode weights; the DMAs drain during the topk phase
                with (
                    tc.tile_pool(name="wdec", bufs=NPREF + 2) as WD,
                    tc.tile_pool(name="dpsum", bufs=2, space="PSUM") as DP,
                    tc.tile_pool(name="msk", bufs=5) as MS,
                    tc.tile_pool(name="outs", bufs=2) as OS,
                ):
                    # ================= threshold =================
                    nc.vector.tensor_reduce(out=tmp1[:], in_=chp_cols[:],
                                            axis=mybir.AxisListType.X, op=Alu.add)
                    psum_reduce(tmp1[:], tmp2[:])
                    nc.vector.tensor_copy(l2[:, L2_W + 2:L2_W + 3], tmp2[:])
                    nc.vector.tensor_reduce(out=tmp1[:], in_=clp_cols[:],
                                            axis=mybir.AxisListType.X, op=Alu.add)
                    psum_reduce(tmp1[:], tmp3[:])
                    nc.vector.tensor_copy(l2[:, L2_W + 3:L2_W + 4], tmp3[:])
                    nc.vector.tensor_copy(l2[:, L2_W:L2_W + 1], lo0[:])
                    nc.vector.tensor_copy(l2[:, L2_W + 1:L2_W + 2], hi0[:])

                    # AllGather input first in the scalar DGE queue (it only
                    # waits on l2's writers), decode-weight prefetch behind it
                    nc.scalar.dma_start(out=g_in[:], in_=l2[:])
                    wd_tiles = {}
                    for fp in range(NPREF):
                        wd2 = WD.tile([128, 2, ACT_DIM], dt.bfloat16, tag="wd")
                        nc.scalar.dma_start(out=wd2[:], in_=wdect[fp])
                        wd_tiles[fp] = wd2
                    nc.gpsimd.collective_compute(
                        "AllGather",
                        Alu.bypass,
                        replica_groups=[list(range(NCORES))],
                        ins=[g_in.opt()],
                        outs=[g_out.opt()],
                    )
                    for c in range(NCORES):
                        nc.sync.dma_start(out=gath[:, c, :], in_=g_out[c])

                    gv = gath[:, :, 0:L2_W]

                    # global bracket / counts from sidecars
                    nc.vector.tensor_reduce(out=lo[:], in_=gath[:, :, L2_W:L2_W + 1],
                                            axis=mybir.AxisListType.XY, op=Alu.max)
                    nc.vector.tensor_reduce(out=hi[:], in_=gath[:, :, L2_W + 1:L2_W + 2],
                                            axis=mybir.AxisListType.XY, op=Alu.min)
                    nc.vector.tensor_reduce(out=chg[:], in_=gath[:, :, L2_W + 2:L2_W + 3],
                                            axis=mybir.AxisListType.XY, op=Alu.add)
                    nc.vector.tensor_reduce(out=clo[:], in_=gath[:, :, L2_W + 3:L2_W + 4],
                                            axis=mybir.AxisListType.XY, op=Alu.add)

                    def count_ge(t_ap, out_ap):
                        ts(cscr_b[:], gv, t_ap, None,
                           op0=Alu.is_ge, op1=Alu.add, accum_out=cp[:])
                        psum_reduce(cp[:], out_ap, add_ap=chg[:])

                    # exact count at the hi bracket; clo stays the (approximate)
                    # sidecar sum -- it only steers the first interpolation
                    count_ge(hi[:], chi[:])

                    # branch-free false position; the hi side converges onto the
                    # target count from below and is the final threshold
                    for it in range(NSECANT):
                        tt(tmp1[:], clo[:], chi[:], op=Alu.subtract)
                        nc.vector.tensor_scalar_max(tmp1[:], tmp1[:], 1.0)
                        nc.vector.reciprocal(inv[:], tmp1[:])
                        ts(tmp2[:], chi[:], -1.0, CTARGET, op0=Alu.mult, op1=Alu.add)
                        tt(tmp2[:], tmp2[:], inv[:], op=Alu.mult)
                        tt(tmp3[:], lo[:], hi[:], op=Alu.subtract)
                        stt(t[:], tmp3[:], tmp2[:], hi[:], op0=Alu.mult, op1=Alu.add)
                        count_ge(t[:], ct[:])
                        ts(pred[:], ct[:], CTARGET, None, op0=Alu.is_ge)
                        ts(npred[:], pred[:], -1.0, 1.0, op0=Alu.mult, op1=Alu.add)
                        tt(tmp1[:], t[:], lo[:], op=Alu.subtract)
                        stt(lo[:], tmp1[:], pred[:], lo[:], op0=Alu.mult, op1=Alu.add)
                        tt(tmp1[:], ct[:], clo[:], op=Alu.subtract)
                        stt(clo[:], tmp1[:], pred[:], clo[:], op0=Alu.mult, op1=Alu.add)
                        tt(tmp1[:], t[:], hi[:], op=Alu.subtract)
                        stt(hi[:], tmp1[:], npred[:], hi[:], op0=Alu.mult, op1=Alu.add)
                        tt(tmp1[:], ct[:], chi[:], op=Alu.subtract)
                        stt(chi[:], tmp1[:], npred[:], chi[:], op0=Alu.mult, op1=Alu.add)

                    # ================= decode =================
                    HA = ACT_DIM // 2  # 384 -- one matmul per PSUM bank
                    pso = [
                        DP.tile([128, 2, 512], dt.float32, tag="dps", name=f"dps{mt}")
                        for mt in range(MT)
                    ]
                    for fp in range(FT // 2):
                        wd2 = wd_tiles.pop(fp, None)
                        if wd2 is None:
                            # alternate the two hardware DGE queues: aggregate
                            # weight-stream bandwidth, not one queue's cap
                            wd2 = WD.tile([128, 2, ACT_DIM], dt.bfloat16, tag="wd")
                            eng = nc.scalar if fp % 2 else nc.sync
                            eng.dma_start(out=wd2[:], in_=wdect[fp])
                        for half in range(2):
                            ft = 2 * fp + half
                            pslice = post[:, ft * ROWS:(ft + 1) * ROWS]
                            mskt = MS.tile([128, ROWS], dt.bfloat16, tag="mskt")
                            stt(mskt[:], pslice, hi[:], pslice, op0=Alu.is_ge, op1=Alu.mult)
                            for mt in range(MT):
                                for h in range(2):
                                    nc.tensor.matmul(
                                        pso[mt][:, h, 0:HA],
                                        mskt[:, mt * 128:(mt + 1) * 128],
                                        wd2[:, half, h * HA:(h + 1) * HA],
                                        start=(ft == 0),
                                        stop=(ft == FT - 1),
                                    )
                    for mt in range(MT):
                        outs = OS.tile([128, ACT_DIM], dt.float32, tag="outs")
                        for h in range(2):
                            nc.vector.tensor_add(
                                outs[:, h * HA:(h + 1) * HA],
                                pso[mt][:, h, 0:HA],
                                bdec_s[:, h * HA:(h + 1) * HA],
                            )
                        nc.sync.dma_start(out=xhat[mt * 128:(mt + 1) * 128, :], in_=outs[:])

    nc.finalize()
    return nc


def _split_fp8(a, nterms):
    """a -> [t0, t1, ...] e4m3 terms with t_i ~ (residual * 16^i); the true
    value is sum_i t_i / 16^i."""
    import ml_dtypes
    f8 = ml_dtypes.float8_e4m3fn
    terms = []
    r = a.astype(np.float32)
    for i in range(nterms):
        t = (r * np.float32(16.0 ** i)).astype(f8)
        terms.append(t)
        r = r - t.astype(np.float32) / np.float32(16.0 ** i)
    return terms


def _prep_inputs(x, W_enc, b_enc, W_dec, b_dec):
    import ml_dtypes
    bf16 = ml_dtypes.bfloat16

    x0T = np.ascontiguousarray(
        (x.astype(np.float32) - b_dec.astype(np.float32)[None, :]).T
    )  # [768, 2048]
    WT = np.ascontiguousarray(W_enc.astype(np.float32).T)  # [768, 16384]

    def wlay(a):  # [768, 16384] -> [FT, 128(p=d), DT, 128(f)]
        return np.ascontiguousarray(
            a.reshape(DT, 128, FT, 128).transpose(2, 1, 0, 3)
        )

    WdT = (
        np.ascontiguousarray(W_dec.astype(np.float32).T).astype(bf16)
        .reshape(FT // 2, 2, 128, ACT_DIM).transpose(0, 2, 1, 3).copy()
    )  # [64, 128, 2, 768]: tile pairs share one DMA line
    bencL = np.ascontiguousarray(b_enc.astype(np.float32).reshape(FT, 128).T)
    bdecB = np.ascontiguousarray(
        np.broadcast_to(b_dec.astype(np.float32)[None, :], (128, ACT_DIM))
    )

    common = {"wdect": WdT, "benc": bencL, "bdec_b": bdecB}
    if ENC == "fp8":
        xterms = _split_fp8(x0T, NXTERM)             # each [768, 2048]
        wterms = _split_fp8(WT * np.float32(WSCALE), NXTERM)  # each [768, 16384]
        # w8: [FT, 128(p=d), NXTERM, DT, 128(f)]
        w8 = np.stack([wlay(t) for t in wterms], axis=2)
        common["w8"] = np.ascontiguousarray(w8)
        xl_ = [t.reshape(DT, 128, BATCH).transpose(1, 0, 2) for t in xterms]
    else:
        import ml_dtypes
        f8 = ml_dtypes.float8_e4m3fn
        XLS = np.float32(256.0)
        WHS = np.float32(BSCALE / 256.0)  # 32; XLS*WHS == BSCALE
        xh = x0T.astype(bf16)
        xl8 = ((x0T - xh.astype(np.float32)) * XLS).astype(f8)
        x8h_full = x0T.astype(f8)
        Wh = WT.astype(bf16)
        Wl8 = ((WT - Wh.astype(np.float32)) * np.float32(BSCALE)).astype(f8)
        Wh8 = (WT * WHS).astype(f8)
        common["wenc_hi"] = wlay(Wh)
        common["wenc_lo8"] = wlay(Wl8)
        common["wenc_hi8"] = wlay(Wh8)

    in_maps = []
    for c in range(NCORES):
        sl = slice(c * ROWS, (c + 1) * ROWS)
        m = dict(common)
        if ENC == "fp8":
            # x8: [128(p=d), NXTERM, DT, ROWS]
            m["x8"] = np.ascontiguousarray(
                np.stack([t[:, :, sl] for t in xl_], axis=1)
            )
        else:
            m["xt_hi"] = np.ascontiguousarray(
                xh[:, sl].reshape(DT, 128, ROWS).transpose(1, 0, 2))
            m["xt_lo8"] = np.ascontiguousarray(
                xl8[:, sl].reshape(DT, 128, ROWS).transpose(1, 0, 2))
            m["x8h"] = np.ascontiguousarray(
                x8h_full[:, sl].reshape(DT, 128, ROWS).transpose(1, 0, 2))
        in_maps.append(m)
    return in_maps


def _ensure_axon_hooks_shim():
    """concourse's trace path imports antenv.axon_hooks, which some images
    lack; install an equivalent module so tracing degrades (or works, when
    the ctypes hook is available) instead of crashing."""
    import sys, types
    try:
        import antenv.axon_hooks  # noqa: F401
        return
    except ImportError:
        pass
    m = types.ModuleType("antenv.axon_hooks")
    state = {"hook": None}
    m.set_axon_ntff_profile_hook = lambda h: state.__setitem__("hook", h)
    m.get_axon_ntff_profile_hook = lambda: state["hook"]
    sys.modules["antenv.axon_hooks"] = m
    try:
        from trn_agent_boot.trn_boot import _ntff_profile_via_ctypes
        hook = _ntff_profile_via_ctypes("/opt/axon/libaxon_pjrt.so")
        if hook is not None:
            m.set_axon_ntff_profile_hook(hook)
    except Exception:
        pass


def kernel(x, W_enc, b_enc, W_dec, b_dec):
    import os
    _ensure_axon_hooks_shim()
    from concourse import bass_utils
    from concourse.bass_utils import run_bass_kernel_spmd

    in_maps = _prep_inputs(x, W_enc, b_enc, W_dec, b_dec)
    nc = build_nc()
    res = None
    if os.environ.get("KERNEL_TRACE"):
        bass_utils.upload_artifacts = lambda d: ""  # no artifact bucket here
        try:
            res = run_bass_kernel_spmd(nc, in_maps, list(range(NCORES)), trace=True)
        except Exception as e:
            print(f"traced run failed ({type(e).__name__}: {e}); retrying untraced")
            res = None
    if res is None:
        res = run_bass_kernel_spmd(nc, in_maps, list(range(NCORES)))
    if res.exec_time_ns is not None:
        print(f"HW exec time: {res.exec_time_ns} ns")
    out = np.concatenate(
        [np.asarray(res.results[c]["xhat"], dtype=np.float32) for c in range(NCORES)],
        axis=0,
    )
    return out
